# revision 17
# baseline (speedup 1.0000x reference)
"""Distributed Bass kernel for nn_Attention_80908593922315 on 8 TRN2 NeuronCores.

Sharding: head-parallel attention (core h owns head h) + spatial-parallel
conv/proj (core j owns flattened-spatial slice [512j, 512j+512)).

v2 changes over the first working version:
  - PE warm-up matmuls at kernel start (HAM un-throttles before the conv).
  - k bias dropped entirely (a per-query constant in the scores cancels in
    softmax), q bias/scale folded on host + ACT evacuation.
  - attention scores computed in 3-chunk packs ([128, 1536] psum, pool
    bufs=2 -> zero PE bubbles) with 3 row-tiled K=32 matmuls per pack on
    rotating 32-row bands.
  - softmax exp split across ACT (exact, table exp) and DVE (Schraudolph
    bitcast-int16 approx) with a greedy per-pack balance; measured end-to-end
    rel-err cost of the approximation is ~3e-3 at a 50% share.
  - reciprocal of the softmax denominators moved to the (otherwise idle)
    GPSIMD engine.
"""

import sys

if "/opt/trn_rl_repo" not in sys.path:
    sys.path.insert(0, "/opt/trn_rl_repo")

import numpy as np
import ml_dtypes

import concourse.bass as bass
import concourse.bacc as bacc
import concourse.tile as tile
import concourse.mybir as mybir
from concourse import bass_utils

BF16 = mybir.dt.bfloat16
F32 = mybir.dt.float32
I16 = mybir.dt.int16
AF = mybir.ActivationFunctionType
ALU = mybir.AluOpType
bf16 = ml_dtypes.bfloat16

NCORES = 8
C = 256
NH = 8
HD = 32
HWD = 16
N = HWD * HWD * HWD  # 4096
NS = N // NCORES  # 512 spatial per core
P = 128
CCH = C // P  # 2 channel chunks
EPS = 1e-5
SCALE = HD ** -0.5
NMB = N // P  # 32 m-chunks
NNB = N // 512  # 8 n-blocks
TAPS = [(a, b, c) for a in range(3) for b in range(3) for c in range(3)]

# Schraudolph bf16 exp: bitcast_bf16(int16(A*x + B)) ~= exp(x)
SCH_A = 128.0 / float(np.log(2.0))
SCH_B = 16250.0

_cache = {}


def _build_graph():
    nc = bacc.Bacc(
        "TRN2",
        target_bir_lowering=False,
        debug=False,
        enable_asserts=True,
        num_devices=NCORES,
    )

    # ---- I/O declarations (per-core shards) ----
    x_pad = nc.dram_tensor("x_pad", [CCH, P, 4 * 18 * 18], BF16, kind="ExternalInput").ap()
    dw_diag = nc.dram_tensor("dw_diag", [CCH, 27, P, P], BF16, kind="ExternalInput").ap()
    b_dw = nc.dram_tensor("b_dw", [CCH, P, 1], F32, kind="ExternalInput").ap()
    wq_d = nc.dram_tensor("wq", [CCH, P, P], BF16, kind="ExternalInput").ap()
    wk_d = nc.dram_tensor("wk", [CCH, P, P], BF16, kind="ExternalInput").ap()
    wv_d = nc.dram_tensor("wv", [CCH, P, HD], BF16, kind="ExternalInput").ap()
    bq_d = nc.dram_tensor("bq", [P, 1], F32, kind="ExternalInput").ap()
    bv_d = nc.dram_tensor("bv", [HD, 1], F32, kind="ExternalInput").ap()
    wproj_d = nc.dram_tensor("wproj", [CCH, P, C], BF16, kind="ExternalInput").ap()
    bproj_d = nc.dram_tensor("bproj", [CCH, P, 1], F32, kind="ExternalInput").ap()
    out_d = nc.dram_tensor("out", [CCH, P, NS], F32, kind="ExternalOutput").ap()

    with tile.TileContext(nc) as tc:
        with tc.tile_pool(name="persist", bufs=1) as persist, \
             tc.tile_pool(name="dram", bufs=1, space="DRAM") as dram, \
             tc.tile_pool(name="work", bufs=4) as work:

            # ---- PE warm-up: ~4us of junk matmuls so HAM un-throttles the
            # PE clock (1.2 -> 2.4 GHz) before the conv needs it. Runs during
            # the input DMAs.
            warm_in = persist.tile([P, 256], BF16, name="warm_in")
            nc.vector.memset(warm_in[:], 0.25)
            # preload the exp activation table while everything else flies
            exp_dummy = work.tile([1, 16], BF16, name="exp_dummy")
            nc.vector.memset(exp_dummy[:], 0.0)
            nc.scalar.activation(exp_dummy, exp_dummy, AF.Exp)
            with tc.tile_pool(name="warm_psum", bufs=1, space="PSUM") as warm_psum:
                wps = warm_psum.tile([P, 256], F32, name="wps")
                for _ in range(32):
                    nc.tensor.matmul(
                        wps, lhsT=warm_in[:, 0:P], rhs=warm_in[:],
                        start=True, stop=True,
                    )

            # ---- load weights/constants (conv inputs first: on critical path) ----
            xp_first = persist.tile([P, CCH, 4 * 18 * 18], BF16, name="xp_first")
            for cc in range(CCH):
                nc.sync.dma_start(xp_first[:, cc], x_pad[cc])
            # per-tap weight DMAs: the first taps land in <1us so the conv
            # matmuls start immediately instead of stalling >3.4us on one
            # 1.8MB transfer (which re-throttled the PE clock)
            dwd_sb = persist.tile([P, CCH, 27, P], BF16, name="dwd_sb")
            for cc in range(CCH):
                for t in range(27):
                    (nc.sync if t % 2 == 0 else nc.gpsimd).dma_start(
                        dwd_sb[:, cc, t], dw_diag[cc, t]
                    )
            bdw_sb = persist.tile([P, CCH], F32, name="bdw_sb")
            for cc in range(CCH):
                nc.sync.dma_start(bdw_sb[:, cc : cc + 1], b_dw[cc])
            wq_sb = persist.tile([P, CCH, P], BF16, name="wq_sb")
            wk_sb = persist.tile([P, CCH, P], BF16, name="wk_sb")
            wv_sb = persist.tile([P, CCH, HD], BF16, name="wv_sb")
            for cc in range(CCH):
                nc.sync.dma_start(wq_sb[:, cc], wq_d[cc])
                nc.sync.dma_start(wk_sb[:, cc], wk_d[cc])
                nc.sync.dma_start(wv_sb[:, cc], wv_d[cc])
            bq_sb = persist.tile([P, 1], F32, name="bq_sb")
            bv_sb = persist.tile([HD, 1], F32, name="bv_sb")
            nc.sync.dma_start(bq_sb[:], bq_d)
            nc.sync.dma_start(bv_sb[:], bv_d)
            wproj_sb = persist.tile([P, CCH, C], BF16, name="wproj_sb")
            for cc in range(CCH):
                nc.sync.dma_start(wproj_sb[:, cc], wproj_d[cc])
            bproj_sb = persist.tile([P, CCH], F32, name="bproj_sb")
            for cc in range(CCH):
                nc.sync.dma_start(bproj_sb[:, cc : cc + 1], bproj_d[cc])
            xp_sb = xp_first

            # ---- phase 1+2: depthwise conv, then one AllGather of (y, stats)
            # Bounce layout [128, 2*514]: both chunks in one partition row so
            # the sbuf->dram DMA moves 2056B contiguous per partition.
            y_sb = persist.tile([P, CCH, 514], BF16, name="y_sb")
            stats_sb = persist.tile([P, CCH, 2], F32, name="stats_sb")
            sq_junk = work.tile([P, NS], BF16, name="sq_junk")
            ag_in = dram.tile([P, CCH * 514], BF16, name="ag_in")
            ag_out = dram.tile([NCORES, P, CCH * 514], BF16, name="ag_out",
                               addr_space="Shared")
            with tc.tile_pool(name="conv_psum", bufs=2, space="PSUM") as conv_psum:
                for cc in range(CCH):
                    ps = conv_psum.tile([P, NS], F32, name="ps_conv")
                    x4 = xp_sb[:, cc].rearrange("p (a b c) -> p a b c", b=18, c=18)
                    for t, (dh, dw_, dd) in enumerate(TAPS):
                        nc.tensor.matmul(
                            ps,
                            lhsT=dwd_sb[:, cc, t],
                            rhs=x4[:, dh : dh + 2, dw_ : dw_ + 16, dd : dd + 16],
                            start=(t == 0),
                            stop=(t == 26),
                        )
                    nc.scalar.activation(
                        y_sb[:, cc, 0:NS], ps, AF.Identity,
                        bias=bdw_sb[:, cc : cc + 1], scale=1.0,
                        accum_out=stats_sb[:, cc, 0:1],
                    )
                    nc.scalar.activation(
                        sq_junk, ps, AF.Square,
                        bias=bdw_sb[:, cc : cc + 1], scale=1.0,
                        accum_out=stats_sb[:, cc, 1:2],
                    )
                    nc.vector.tensor_copy(y_sb[:, cc, NS : NS + 2], stats_sb[:, cc])
                    # per-chunk bounce: chunk 0's half hides under chunk-1 conv
                    (nc.sync if cc == 0 else nc.gpsimd).dma_start(
                        ag_in[:, cc * 514 : (cc + 1) * 514], y_sb[:, cc, :]
                    )
            nc.gpsimd.collective_compute(
                "AllGather",
                ALU.bypass,
                replica_groups=[list(range(NCORES))],
                ins=[ag_in[:].opt()],
                outs=[ag_out[:].opt()],
            )

            y_full = persist.tile([P, CCH, N], BF16, name="y_full")
            stats_g = work.tile([P, CCH, NCORES, 2], BF16, name="stats_g")
            ssum = work.tile([P, CCH, 2], F32, name="ssum")
            ago = ag_out[:].rearrange("r p (q f) -> p q r f", q=CCH)  # [128,2,8,514]
            # stats first (tiny) so the weight-fold chain runs during the
            # 2MB y_full loads instead of queueing behind them
            for cc in range(CCH):
                nc.sync.dma_start(stats_g[:, cc], ago[:, cc, :, NS : NS + 2])
            # per-rank y loads so early n-blocks' qkv/attention can start
            # before the full 2MB has landed
            for r in range(NCORES):
                for cc in range(CCH):
                    (nc.sync if r % 2 == 0 else nc.gpsimd).dma_start(
                        y_full[:, cc, r * NS : (r + 1) * NS],
                        ago[:, cc, r, 0:NS],
                    )
            for cc in range(CCH):
                nc.vector.reduce_sum(
                    ssum[:, cc],
                    stats_g[:, cc].rearrange("p r s -> p s r"),
                    axis=mybir.AxisListType.X,
                )

            mu = persist.tile([P, CCH], F32, name="mu")
            rstd = persist.tile([P, CCH], F32, name="rstd")
            t_a = work.tile([P, 1], F32, name="t_a")
            t_b = work.tile([P, 1], F32, name="t_b")
            t_c = work.tile([P, 1], F32, name="t_c")
            for cc in range(CCH):
                mcc = mu[:, cc : cc + 1]
                rcc = rstd[:, cc : cc + 1]
                nc.vector.tensor_scalar_mul(mcc, ssum[:, cc, 0:1], 1.0 / N)
                nc.vector.tensor_scalar_mul(t_a, ssum[:, cc, 1:2], 1.0 / N)  # E[y^2]
                nc.vector.tensor_tensor(t_b, mcc, mcc, ALU.mult)  # mu^2
                nc.vector.tensor_tensor(t_a, t_a, t_b, ALU.subtract)  # var
                nc.vector.tensor_scalar_add(t_a, t_a, EPS)  # var+eps
                # quake rsqrt seed (no ACT table set needed) + 2 Newton steps
                rci = rcc.bitcast(mybir.dt.int32)
                nc.vector.tensor_scalar(
                    rci, t_a.bitcast(mybir.dt.int32), 1, -1,
                    op0=ALU.arith_shift_right, op1=ALU.bitwise_xor,
                )
                nc.vector.tensor_scalar_add(rci, rci, 0x5F3759DF + 1)
                for _ in range(2):
                    nc.vector.tensor_tensor(t_b, rcc, rcc, ALU.mult)
                    nc.vector.tensor_tensor(t_c, t_a, t_b, ALU.mult)
                    nc.vector.tensor_scalar(
                        t_c, t_c, -0.5, 1.5, op0=ALU.mult, op1=ALU.add
                    )
                    nc.vector.tensor_tensor(rcc, rcc, t_c, ALU.mult)

            # fold norm scale into qkv weights (wq/bq carry SCALE from host)
            wq_s = persist.tile([P, CCH, P], BF16, name="wq_s")
            wk_s = persist.tile([P, CCH, P], BF16, name="wk_s")
            wv_s = persist.tile([P, CCH, HD], BF16, name="wv_s")
            mu_bf = work.tile([P, CCH], BF16, name="mu_bf")
            nc.vector.tensor_copy(mu_bf[:], mu[:])
            for cc in range(CCH):
                nc.vector.tensor_scalar_mul(wq_s[:, cc], wq_sb[:, cc], rstd[:, cc : cc + 1])
                nc.vector.tensor_scalar_mul(wk_s[:, cc], wk_sb[:, cc], rstd[:, cc : cc + 1])
                nc.vector.tensor_scalar_mul(wv_s[:, cc], wv_sb[:, cc], rstd[:, cc : cc + 1])

            # effective biases: b' = b - W_scaled @ mu  (no k bias: a
            # per-query constant in the scores cancels in softmax)
            bq_eff = persist.tile([P, 1], F32, name="bq_eff")
            bv_eff = persist.tile([HD, 1], F32, name="bv_eff")
            with tc.tile_pool(name="bias_psum", bufs=1, space="PSUM") as bias_psum:
                pq = bias_psum.tile([P, 1], F32, name="pq")
                pv = bias_psum.tile([P, 1], F32, name="pv")
                for cc in range(CCH):
                    nc.tensor.matmul(pq, lhsT=wq_s[:, cc], rhs=mu_bf[:, cc : cc + 1],
                                     start=(cc == 0), stop=(cc == CCH - 1))
                    nc.tensor.matmul(pv[0:HD], lhsT=wv_s[:, cc], rhs=mu_bf[:, cc : cc + 1],
                                     start=(cc == 0), stop=(cc == CCH - 1))
                nc.vector.tensor_tensor(bq_eff, bq_sb, pq, ALU.subtract)
                nc.vector.tensor_tensor(bv_eff, bv_sb, pv[0:HD], ALU.subtract)

            # ---- phase 3: qkv ----
            qT = persist.tile([P, N], BF16, name="qT")
            kT = persist.tile([P, N], BF16, name="kT")
            v_sb = persist.tile([P, NMB * (HD + 1)], BF16, name="v_sb")
            nc.gpsimd.memset(v_sb[:], 1.0)  # ones column default
            with tc.tile_pool(name="mm_psum", bufs=3, space="PSUM") as mm_psum:
                # per n-block (= per source rank) so attention inputs become
                # ready progressively as each rank's y slab lands
                for nb in range(NNB):
                    for mb in range(4 * nb, 4 * nb + 4):
                        psv = mm_psum.tile([P, 512], F32, name="ps_mm")
                        for cc in range(CCH):
                            nc.tensor.matmul(
                                psv[:, 0:HD],
                                lhsT=y_full[:, cc, mb * P : (mb + 1) * P],
                                rhs=wv_s[:, cc],
                                start=(cc == 0), stop=(cc == CCH - 1),
                            )
                        nc.vector.tensor_copy(
                            v_sb[:, mb * (HD + 1) : mb * (HD + 1) + HD],
                            psv[:, 0:HD],
                        )
                    psq = mm_psum.tile([P, 512], F32, name="ps_mm")
                    for cc in range(CCH):
                        nc.tensor.matmul(
                            psq, lhsT=wq_s[:, cc],
                            rhs=y_full[:, cc, nb * 512 : (nb + 1) * 512],
                            start=(cc == 0), stop=(cc == CCH - 1),
                        )
                    nc.scalar.activation(
                        qT[:, nb * 512 : (nb + 1) * 512], psq, AF.Identity,
                        bias=bq_eff, scale=1.0,
                    )
                    psk = mm_psum.tile([P, 512], F32, name="ps_mm")
                    for cc in range(CCH):
                        nc.tensor.matmul(
                            psk, lhsT=wk_s[:, cc],
                            rhs=y_full[:, cc, nb * 512 : (nb + 1) * 512],
                            start=(cc == 0), stop=(cc == CCH - 1),
                        )
                    nc.vector.tensor_copy(kT[:, nb * 512 : (nb + 1) * 512], psk)

            # ---- phase 4: attention ----
            # Per n-block b: scoresT[m, n] in 3-chunk packs ([128, 1536] psum,
            # 3 row-tiled K=32 matmuls on rotating 32-row bands) -> exp split
            # ACT (exact) / DVE (Schraudolph int16 bitcast) -> AV accumulated
            # in psum with m-chunk parity col-packed to partitions 0-32/64-96
            # (row 32 = sum of exp = softmax denominator via the ones column).
            numer4 = persist.tile([P, 2, 512], F32, name="numer4")
            den4 = persist.tile([P, 2, 512], F32, name="den4")
            recip4 = persist.tile([P, 2, 512], F32, name="recip4")
            nc.gpsimd.memset(den4[:], 1.0)
            out4 = persist.tile([P, 2, 512], BF16, name="out4")
            bcast_sb = persist.tile([P, 512], F32, name="bcast_sb")
            a2a_in = dram.tile([NCORES, HD, NS], BF16, name="a2a_in")
            a2a_out = dram.tile([NCORES, HD, NS], BF16, name="a2a_out")
            bv4 = persist.tile([P, 1], F32, name="bv4")
            for i in range(4):
                nc.vector.tensor_copy(bv4[32 * i : 32 * i + 32], bv_eff)

            # pack layout: chunks [3p, 3p+1, 3p+2] (last pack has 2);
            # greedy ACT/DVE split at 512-col granularity per pack
            packs = [list(range(i, min(i + 3, NMB))) for i in range(0, NMB, 3)]
            ACT_NS0, ACT_NSC = 150.0, 1.0 / 1.2288   # overhead ns, ns/col
            DVE_NS0, DVE_NSC = 200.0, 1.0 / 0.96
            DVE_BLK_FIXED = 2400.0  # per-block evac/combine/norm on DVE

            with tc.tile_pool(name="sc_psum", bufs=2, space="PSUM") as sc_psum, \
                 tc.tile_pool(name="av_psum", bufs=2, space="PSUM") as av_psum, \
                 tc.tile_pool(name="exp_pool", bufs=6) as exp_pool:
                band = 0
                act_ns, dve_ns = 0.0, 0.0
                av_t = {}        # block -> av psum tile
                exd = {}         # block -> {pack_idx: ex tile}
                next_pair = {}   # block -> next uncovered m-chunk pair

                def finish_block(b):
                    # evacuate + combine halves (frees the av bank quickly);
                    # a DVE op may read at most ONE psum operand -> stage the
                    # second column band through SBUF first
                    av = av_t.pop(b)
                    exd.pop(b)
                    bh = 32 * (b % 4)
                    g = b // 4
                    tmp33 = work.tile([HD + 1, 512], F32, name="tmp33")
                    nc.vector.tensor_copy(tmp33, av[64 : 64 + HD + 1, :])
                    nc.vector.tensor_tensor(
                        numer4[bh : bh + HD, g, :],
                        av[0:HD, :], tmp33[0:HD, :], ALU.add,
                    )
                    nc.vector.tensor_tensor(
                        den4[bh : bh + 1, g, :],
                        av[HD : HD + 1, :], tmp33[HD : HD + 1, :], ALU.add,
                    )
                    if b % 4 == 3:
                        # normalize this group of 4 blocks (group 0 overlaps
                        # with blocks 4-7 compute); denominator rows sit at
                        # quadrant leaders {0,32,64,96}: fast-approx
                        # reciprocal (1 DVE op, ~18 bits — plenty here),
                        # broadcast + multiply + bias on DVE.
                        g_ = b // 4
                        nc.vector.reciprocal_approx_fast(
                            recip4[:, g_, :], den4[:, g_, :]
                        )
                        nc.vector.stream_shuffle(
                            bcast_sb[:], recip4[:, g_, :], mask=[0] * 32
                        )
                        nc.vector.tensor_tensor(
                            out4[:, g_, :], numer4[:, g_, :], bcast_sb, ALU.mult
                        )
                        nc.vector.tensor_scalar_add(
                            out4[:, g_, :], out4[:, g_, :], bv4
                        )
                        for j in range(4 * g_, 4 * g_ + 4):
                            nc.sync.dma_start(
                                a2a_in[j],
                                out4[32 * (j % 4) : 32 * (j % 4) + HD, j // 4, :],
                            )

                def emit_tail(b, pi, pk, sc):
                    # exp + AV for a pack whose scores were emitted one
                    # iteration earlier (software pipelining: the next pack's
                    # score matmuls sit AHEAD of these AV matmuls in PE
                    # program order, so the PE streams scores while ACT/DVE
                    # do this pack's exp)
                    nonlocal act_ns, dve_ns
                    if pi == 0:
                        av_t[b] = av_psum.tile([97, 512], F32, name="av")
                        exd[b] = {}
                        next_pair[b] = 0
                        dve_ns += DVE_BLK_FIXED
                    ncols = 512 * len(pk)
                    # choose ACT's share (bank-aligned) to balance engines
                    best, best_a = None, ncols
                    for a in range(0, ncols + 1, 512):
                        fin = max(
                            act_ns + (ACT_NS0 + a * ACT_NSC if a else 0.0),
                            dve_ns + (DVE_NS0 + (ncols - a) * DVE_NSC
                                      if a < ncols else 0.0),
                        )
                        if best is None or fin < best:
                            best, best_a = fin, a
                    a = best_a
                    # SEPARATE output tiles per engine: a shared tile would
                    # WAW-serialize the DVE piece behind the ACT piece, which
                    # delays freeing the score psum bank and bubbles the pipe
                    ex_a = ex_d = None
                    if a > 0:
                        ex_a = exp_pool.tile([P, 1536], BF16, name="ex_a")
                        nc.scalar.activation(ex_a[:, 0:a], sc[:, 0:a], AF.Exp)
                        act_ns += ACT_NS0 + a * ACT_NSC
                    if a < ncols:
                        ex_d = exp_pool.tile([P, 1536], BF16, name="ex_d")
                        exi = ex_d[:, 0 : ncols - a].bitcast(I16)
                        nc.vector.tensor_scalar(
                            exi, sc[:, a:ncols], SCH_A, SCH_B,
                            op0=ALU.mult, op1=ALU.add,
                        )
                        dve_ns += DVE_NS0 + (ncols - a) * DVE_NSC
                    exd[b][pi] = (ex_a, ex_d, a)
                    # AV for every m-chunk pair fully covered by now
                    av = av_t[b]
                    done = pk[-1]
                    while next_pair[b] * 2 + 1 <= done:
                        for cm in (2 * next_pair[b], 2 * next_pair[b] + 1):
                            rr = cm % 2
                            pea, ped, pa = exd[b][cm // 3]
                            off = (cm % 3) * 512
                            if off < pa:
                                src = pea[:, off : off + 512]
                            else:
                                src = ped[:, off - pa : off - pa + 512]
                            nc.tensor.matmul(
                                av[64 * rr : 64 * rr + HD + 1, :],
                                lhsT=v_sb[:, cm * (HD + 1) : (cm + 1) * (HD + 1)],
                                rhs=src,
                                start=(cm < 2),
                                stop=(cm >= NMB - 2),
                                tile_position=(0, 64 * rr),
                                skip_group_check=True,
                            )
                        next_pair[b] += 1
                    if pi == len(packs) - 1:
                        finish_block(b)

                pend = None
                for b in range(NNB):
                    for pi, pk in enumerate(packs):
                        sc = sc_psum.tile([P, 1536], F32, name="sc")
                        for j, cm in enumerate(pk):
                            nc.tensor.matmul(
                                sc[:, j * 512 : (j + 1) * 512],
                                lhsT=kT[32 * band : 32 * band + 32,
                                        cm * P : (cm + 1) * P],
                                rhs=qT[32 * band : 32 * band + 32,
                                       b * 512 : (b + 1) * 512],
                                start=True, stop=True,
                                tile_position=(32 * band, 0),
                            )
                            band = (band + 1) % 4
                        if pend is not None:
                            emit_tail(*pend)
                        pend = (b, pi, pk, sc)
                emit_tail(*pend)

            # ---- phase 5: all-to-all + projection ----
            nc.gpsimd.collective_compute(
                "AllToAll",
                ALU.bypass,
                replica_groups=[list(range(NCORES))],
                ins=[a2a_in[:].opt()],
                outs=[a2a_out[:].opt()],
            )
            # keep the PE busy through the all-to-all so HAM doesn't
            # re-throttle the clock before the projection matmuls
            with tc.tile_pool(name="warm2_psum", bufs=1, space="PSUM") as warm2:
                wps2 = warm2.tile([P, 256], F32, name="wps2")
                for _ in range(40):
                    nc.tensor.matmul(
                        wps2, lhsT=warm_in[:, 0:P], rhs=warm_in[:],
                        start=True, stop=True,
                    )
            cat = a2a_out[:].rearrange("h d f -> (h d) f")  # [256, 512]
            c_sb = persist.tile([P, CCH, NS], BF16, name="c_sb")
            out_sb = persist.tile([P, CCH, NS], F32, name="out_sb")
            with tc.tile_pool(name="proj_psum", bufs=2, space="PSUM") as proj_psum:
                # split spatial in half so the second half's DMA overlaps the
                # first half's matmuls (the PE is cold here; keep it short)
                for h2 in range(2):
                    s0, s1 = h2 * 256, (h2 + 1) * 256
                    for cc in range(CCH):
                        nc.sync.dma_start(
                            c_sb[:, cc, s0:s1], cat[cc * P : (cc + 1) * P, s0:s1]
                        )
                    for ob in range(CCH):
                        psp = proj_psum.tile([P, 256], F32, name="ps_proj")
                        for cc in range(CCH):
                            nc.tensor.matmul(
                                psp,
                                lhsT=wproj_sb[:, cc, ob * P : (ob + 1) * P],
                                rhs=c_sb[:, cc, s0:s1],
                                start=(cc == 0), stop=(cc == CCH - 1),
                            )
                        nc.scalar.activation(
                            out_sb[:, ob, s0:s1], psp, AF.Identity,
                            bias=bproj_sb[:, ob : ob + 1], scale=1.0,
                        )
                        nc.sync.dma_start(out_d[ob, :, s0:s1], out_sb[:, ob, s0:s1])

    nc.compile()
    return nc


def _host_prep(x, w_dw, b_dw, w_qkv, b_qkv, w_proj, b_proj):
    """Build per-core in_maps from full inputs."""
    x = np.asarray(x, np.float32)
    w_dw = np.asarray(w_dw, np.float32)
    b_dw = np.asarray(b_dw, np.float32)
    w_qkv = np.asarray(w_qkv, np.float32)
    b_qkv = np.asarray(b_qkv, np.float32)
    w_proj = np.asarray(w_proj, np.float32)
    b_proj = np.asarray(b_proj, np.float32)

    xs = x[0]  # [C, 16, 16, 16]
    # diag conv weights: [CCH, 27, P, P]
    dw_diag = np.zeros((CCH, 27, P, P), np.float32)
    for cc in range(CCH):
        for t, (dh, dw_, dd) in enumerate(TAPS):
            np.fill_diagonal(dw_diag[cc, t], w_dw[cc * P : (cc + 1) * P, 0, dh, dw_, dd])
    dw_diag = dw_diag.astype(bf16)
    b_dw_s = b_dw.reshape(CCH, P, 1)

    wproj_t = np.ascontiguousarray(w_proj.T).reshape(CCH, P, C).astype(bf16)
    bproj_s = b_proj.reshape(CCH, P, 1)

    in_maps = []
    for h in range(NCORES):
        # padded x slab: global h rows 2h-1 .. 2h+2, padded w/d
        xp = np.zeros((C, 4, 18, 18), np.float32)
        for hl in range(4):
            hg = 2 * h - 1 + hl
            if 0 <= hg < HWD:
                xp[:, hl, 1:17, 1:17] = xs[:, hg]
        xp = xp.reshape(CCH, P, 4 * 18 * 18).astype(bf16)

        # q weights/bias carry the 1/sqrt(hd) scale from the host
        wq_h = w_qkv[h * HD : (h + 1) * HD] * SCALE  # [32, 256]
        wk_h = w_qkv[C + h * HD : C + (h + 1) * HD]
        wv_h = w_qkv[2 * C + h * HD : 2 * C + (h + 1) * HD]
        wq_rep = np.tile(wq_h.T, (1, 4)).reshape(C, P)  # [256, 128]
        wk_rep = np.tile(wk_h.T, (1, 4)).reshape(C, P)
        in_maps.append({
            "x_pad": xp,
            "dw_diag": dw_diag,
            "b_dw": b_dw_s,
            "wq": wq_rep.reshape(CCH, P, P).astype(bf16),
            "wk": wk_rep.reshape(CCH, P, P).astype(bf16),
            "wv": np.ascontiguousarray(wv_h.T).reshape(CCH, P, HD).astype(bf16),
            "bq": (np.tile(b_qkv[h * HD : (h + 1) * HD], 4) * SCALE)
                  .reshape(P, 1).astype(np.float32),
            "bv": b_qkv[2 * C + h * HD : 2 * C + (h + 1) * HD].reshape(HD, 1).astype(np.float32),
            "wproj": wproj_t,
            "bproj": bproj_s,
        })
    return in_maps


def kernel(**inputs):
    if "nc" not in _cache:
        _cache["nc"] = _build_graph()
    nc = _cache["nc"]
    in_maps = _host_prep(**inputs)
    res = bass_utils.run_bass_kernel_spmd(nc, in_maps, core_ids=list(range(NCORES)))
    slices = [res.results[j]["out"].reshape(C, NS) for j in range(NCORES)]
    full = np.concatenate(slices, axis=1)  # [256, 4096]
    return full.reshape(1, C, HWD, HWD, HWD).astype(np.float32)


if __name__ == "__main__":
    nc = _build_graph()
    print("graph built + compiled OK")


# revision 20
# speedup vs baseline: 1.1817x; 1.1817x over previous
"""Distributed Bass kernel for nn_Attention_80908593922315 on 8 TRN2 NeuronCores.

Sharding: head-parallel attention (core h owns head h) + spatial-parallel
conv/proj (core j owns flattened-spatial slice [512j, 512j+512)).

v2 changes over the first working version:
  - PE warm-up matmuls at kernel start (HAM un-throttles before the conv).
  - k bias dropped entirely (a per-query constant in the scores cancels in
    softmax), q bias/scale folded on host + ACT evacuation.
  - attention scores computed in 3-chunk packs ([128, 1536] psum, pool
    bufs=2 -> zero PE bubbles) with 3 row-tiled K=32 matmuls per pack on
    rotating 32-row bands.
  - softmax exp split across ACT (exact, table exp) and DVE (Schraudolph
    bitcast-int16 approx) with a greedy per-pack balance; measured end-to-end
    rel-err cost of the approximation is ~3e-3 at a 50% share.
  - reciprocal of the softmax denominators moved to the (otherwise idle)
    GPSIMD engine.
"""

import sys

if "/opt/trn_rl_repo" not in sys.path:
    sys.path.insert(0, "/opt/trn_rl_repo")

import numpy as np
import ml_dtypes

import concourse.bass as bass
import concourse.bacc as bacc
import concourse.tile as tile
import concourse.mybir as mybir
from concourse import bass_utils

BF16 = mybir.dt.bfloat16
F32 = mybir.dt.float32
I16 = mybir.dt.int16
AF = mybir.ActivationFunctionType
ALU = mybir.AluOpType
bf16 = ml_dtypes.bfloat16

NCORES = 8
C = 256
NH = 8
HD = 32
HWD = 16
N = HWD * HWD * HWD  # 4096
NS = N // NCORES  # 512 spatial per core
P = 128
CCH = C // P  # 2 channel chunks
EPS = 1e-5
SCALE = HD ** -0.5
NMB = N // P  # 32 m-chunks
NNB = N // 512  # 8 n-blocks
TAPS = [(a, b, c) for a in range(3) for b in range(3) for c in range(3)]

# Schraudolph bf16 exp: bitcast_bf16(int16(A*x + B)) ~= exp(x)
SCH_A = 128.0 / float(np.log(2.0))
SCH_B = 16250.0

_cache = {}


def _build_graph():
    nc = bacc.Bacc(
        "TRN2",
        target_bir_lowering=False,
        debug=False,
        enable_asserts=True,
        num_devices=NCORES,
    )

    # ---- I/O declarations (per-core shards) ----
    x_pad = nc.dram_tensor("x_pad", [CCH, P, 4 * 18 * 18], BF16, kind="ExternalInput").ap()
    dw_diag = nc.dram_tensor("dw_diag", [CCH, 27, P, P], BF16, kind="ExternalInput").ap()
    b_dw = nc.dram_tensor("b_dw", [CCH, P, 1], F32, kind="ExternalInput").ap()
    wq_d = nc.dram_tensor("wq", [CCH, P, P], BF16, kind="ExternalInput").ap()
    wk_d = nc.dram_tensor("wk", [CCH, P, P], BF16, kind="ExternalInput").ap()
    wv_d = nc.dram_tensor("wv", [CCH, P, HD], BF16, kind="ExternalInput").ap()
    bq_d = nc.dram_tensor("bq", [P, 1], F32, kind="ExternalInput").ap()
    bv_d = nc.dram_tensor("bv", [HD, 1], F32, kind="ExternalInput").ap()
    wproj_d = nc.dram_tensor("wproj", [CCH, P, C], BF16, kind="ExternalInput").ap()
    bproj_d = nc.dram_tensor("bproj", [CCH, P, 1], F32, kind="ExternalInput").ap()
    out_d = nc.dram_tensor("out", [CCH, P, NS], F32, kind="ExternalOutput").ap()

    with tile.TileContext(nc) as tc:
        with tc.tile_pool(name="persist", bufs=1) as persist, \
             tc.tile_pool(name="dram", bufs=1, space="DRAM") as dram, \
             tc.tile_pool(name="work", bufs=4) as work:

            # ---- PE warm-up: ~4us of junk matmuls so HAM un-throttles the
            # PE clock (1.2 -> 2.4 GHz) before the conv needs it. Runs during
            # the input DMAs.
            warm_in = persist.tile([P, 256], BF16, name="warm_in")
            nc.vector.memset(warm_in[:], 0.25)
            # preload the exp activation table while everything else flies
            exp_dummy = work.tile([1, 16], BF16, name="exp_dummy")
            nc.vector.memset(exp_dummy[:], 0.0)
            nc.scalar.activation(exp_dummy, exp_dummy, AF.Exp)
            with tc.tile_pool(name="warm_psum", bufs=1, space="PSUM") as warm_psum:
                wps = warm_psum.tile([P, 256], F32, name="wps")
                for _ in range(32):
                    nc.tensor.matmul(
                        wps, lhsT=warm_in[:, 0:P], rhs=warm_in[:],
                        start=True, stop=True,
                    )

            # ---- load weights/constants (conv inputs first: on critical path) ----
            xp_first = persist.tile([P, CCH, 4 * 18 * 18], BF16, name="xp_first")
            for cc in range(CCH):
                nc.sync.dma_start(xp_first[:, cc], x_pad[cc])
            # conv weights in 4-tap groups: small enough that the first
            # group lands in ~1us (so conv matmuls start immediately), big
            # enough that DMA descriptor-issue overhead (~600ns each) does
            # not pace the conv
            dwd_sb = persist.tile([P, CCH, 27, P], BF16, name="dwd_sb")
            for cc in range(CCH):
                for gi, t0 in enumerate(range(0, 27, 4)):
                    t1 = min(t0 + 4, 27)
                    (nc.sync if gi % 2 == 0 else nc.gpsimd).dma_start(
                        dwd_sb[:, cc, t0:t1],
                        dw_diag[cc, t0:t1].rearrange("t p q -> p t q"),
                    )
            bdw_sb = persist.tile([P, CCH], F32, name="bdw_sb")
            for cc in range(CCH):
                nc.sync.dma_start(bdw_sb[:, cc : cc + 1], b_dw[cc])
            wq_sb = persist.tile([P, CCH, P], BF16, name="wq_sb")
            wk_sb = persist.tile([P, CCH, P], BF16, name="wk_sb")
            wv_sb = persist.tile([P, CCH, HD], BF16, name="wv_sb")
            for cc in range(CCH):
                nc.sync.dma_start(wq_sb[:, cc], wq_d[cc])
                nc.sync.dma_start(wk_sb[:, cc], wk_d[cc])
                nc.sync.dma_start(wv_sb[:, cc], wv_d[cc])
            bq_sb = persist.tile([P, 1], F32, name="bq_sb")
            bv_sb = persist.tile([HD, 1], F32, name="bv_sb")
            nc.sync.dma_start(bq_sb[:], bq_d)
            nc.sync.dma_start(bv_sb[:], bv_d)
            wproj_sb = persist.tile([P, CCH, C], BF16, name="wproj_sb")
            for cc in range(CCH):
                nc.sync.dma_start(wproj_sb[:, cc], wproj_d[cc])
            bproj_sb = persist.tile([P, CCH], F32, name="bproj_sb")
            for cc in range(CCH):
                nc.sync.dma_start(bproj_sb[:, cc : cc + 1], bproj_d[cc])
            xp_sb = xp_first

            # ---- phase 1+2: depthwise conv, then one AllGather of (y, stats)
            # Bounce layout [128, 2*514]: both chunks in one partition row so
            # the sbuf->dram DMA moves 2056B contiguous per partition.
            y_sb = persist.tile([P, CCH, 514], BF16, name="y_sb")
            stats_sb = persist.tile([P, CCH, 2], F32, name="stats_sb")
            sq_junk = work.tile([P, NS], BF16, name="sq_junk")
            ag_in = dram.tile([P, CCH * 514], BF16, name="ag_in")
            ag_out = dram.tile([NCORES, P, CCH * 514], BF16, name="ag_out",
                               addr_space="Shared")
            with tc.tile_pool(name="conv_psum", bufs=2, space="PSUM") as conv_psum:
                for cc in range(CCH):
                    ps = conv_psum.tile([P, NS], F32, name="ps_conv")
                    x4 = xp_sb[:, cc].rearrange("p (a b c) -> p a b c", b=18, c=18)
                    for t, (dh, dw_, dd) in enumerate(TAPS):
                        nc.tensor.matmul(
                            ps,
                            lhsT=dwd_sb[:, cc, t],
                            rhs=x4[:, dh : dh + 2, dw_ : dw_ + 16, dd : dd + 16],
                            start=(t == 0),
                            stop=(t == 26),
                        )
                    nc.scalar.activation(
                        y_sb[:, cc, 0:NS], ps, AF.Identity,
                        bias=bdw_sb[:, cc : cc + 1], scale=1.0,
                        accum_out=stats_sb[:, cc, 0:1],
                    )
                    nc.scalar.activation(
                        sq_junk, ps, AF.Square,
                        bias=bdw_sb[:, cc : cc + 1], scale=1.0,
                        accum_out=stats_sb[:, cc, 1:2],
                    )
                    nc.vector.tensor_copy(y_sb[:, cc, NS : NS + 2], stats_sb[:, cc])
                    # per-chunk bounce: chunk 0's half hides under chunk-1 conv
                    (nc.sync if cc == 0 else nc.gpsimd).dma_start(
                        ag_in[:, cc * 514 : (cc + 1) * 514], y_sb[:, cc, :]
                    )
            nc.gpsimd.collective_compute(
                "AllGather",
                ALU.bypass,
                replica_groups=[list(range(NCORES))],
                ins=[ag_in[:].opt()],
                outs=[ag_out[:].opt()],
            )

            y_full = persist.tile([P, CCH, N], BF16, name="y_full")
            stats_g = work.tile([P, CCH, NCORES, 2], BF16, name="stats_g")
            ssum = work.tile([P, CCH, 2], F32, name="ssum")
            ago = ag_out[:].rearrange("r p (q f) -> p q r f", q=CCH)  # [128,2,8,514]
            # stats first (tiny) so the weight-fold chain runs during the
            # 2MB y_full loads instead of queueing behind them
            for cc in range(CCH):
                nc.sync.dma_start(stats_g[:, cc], ago[:, cc, :, NS : NS + 2])
            # per-rank y loads so early n-blocks' qkv/attention can start
            # before the full 2MB has landed
            for r in range(NCORES):
                for cc in range(CCH):
                    (nc.sync if r % 2 == 0 else nc.gpsimd).dma_start(
                        y_full[:, cc, r * NS : (r + 1) * NS],
                        ago[:, cc, r, 0:NS],
                    )
            for cc in range(CCH):
                nc.vector.reduce_sum(
                    ssum[:, cc],
                    stats_g[:, cc].rearrange("p r s -> p s r"),
                    axis=mybir.AxisListType.X,
                )

            mu = persist.tile([P, CCH], F32, name="mu")
            rstd = persist.tile([P, CCH], F32, name="rstd")
            t_a = work.tile([P, 1], F32, name="t_a")
            t_b = work.tile([P, 1], F32, name="t_b")
            t_c = work.tile([P, 1], F32, name="t_c")
            for cc in range(CCH):
                mcc = mu[:, cc : cc + 1]
                rcc = rstd[:, cc : cc + 1]
                nc.vector.tensor_scalar_mul(mcc, ssum[:, cc, 0:1], 1.0 / N)
                nc.vector.tensor_scalar_mul(t_a, ssum[:, cc, 1:2], 1.0 / N)  # E[y^2]
                nc.vector.tensor_tensor(t_b, mcc, mcc, ALU.mult)  # mu^2
                nc.vector.tensor_tensor(t_a, t_a, t_b, ALU.subtract)  # var
                nc.vector.tensor_scalar_add(t_a, t_a, EPS)  # var+eps
                # quake rsqrt seed (no ACT table set needed) + 2 Newton steps
                rci = rcc.bitcast(mybir.dt.int32)
                nc.vector.tensor_scalar(
                    rci, t_a.bitcast(mybir.dt.int32), 1, -1,
                    op0=ALU.arith_shift_right, op1=ALU.bitwise_xor,
                )
                nc.vector.tensor_scalar_add(rci, rci, 0x5F3759DF + 1)
                for _ in range(2):
                    nc.vector.tensor_tensor(t_b, rcc, rcc, ALU.mult)
                    nc.vector.tensor_tensor(t_c, t_a, t_b, ALU.mult)
                    nc.vector.tensor_scalar(
                        t_c, t_c, -0.5, 1.5, op0=ALU.mult, op1=ALU.add
                    )
                    nc.vector.tensor_tensor(rcc, rcc, t_c, ALU.mult)

            # fold norm scale into qkv weights (wq/bq carry SCALE from host)
            wq_s = persist.tile([P, CCH, P], BF16, name="wq_s")
            wk_s = persist.tile([P, CCH, P], BF16, name="wk_s")
            wv_s = persist.tile([P, CCH, HD], BF16, name="wv_s")
            mu_bf = work.tile([P, CCH], BF16, name="mu_bf")
            nc.vector.tensor_copy(mu_bf[:], mu[:])
            for cc in range(CCH):
                nc.vector.tensor_scalar_mul(wq_s[:, cc], wq_sb[:, cc], rstd[:, cc : cc + 1])
                nc.vector.tensor_scalar_mul(wk_s[:, cc], wk_sb[:, cc], rstd[:, cc : cc + 1])
                nc.vector.tensor_scalar_mul(wv_s[:, cc], wv_sb[:, cc], rstd[:, cc : cc + 1])

            # effective biases: b' = b - W_scaled @ mu  (no k bias: a
            # per-query constant in the scores cancels in softmax)
            bq_eff = persist.tile([P, 1], F32, name="bq_eff")
            bv_eff = persist.tile([HD, 1], F32, name="bv_eff")
            with tc.tile_pool(name="bias_psum", bufs=1, space="PSUM") as bias_psum:
                pq = bias_psum.tile([P, 1], F32, name="pq")
                pv = bias_psum.tile([P, 1], F32, name="pv")
                for cc in range(CCH):
                    nc.tensor.matmul(pq, lhsT=wq_s[:, cc], rhs=mu_bf[:, cc : cc + 1],
                                     start=(cc == 0), stop=(cc == CCH - 1))
                    nc.tensor.matmul(pv[0:HD], lhsT=wv_s[:, cc], rhs=mu_bf[:, cc : cc + 1],
                                     start=(cc == 0), stop=(cc == CCH - 1))
                nc.vector.tensor_tensor(bq_eff, bq_sb, pq, ALU.subtract)
                nc.vector.tensor_tensor(bv_eff, bv_sb, pv[0:HD], ALU.subtract)

            # ---- phase 3: qkv ----
            qT = persist.tile([P, N], BF16, name="qT")
            kT = persist.tile([P, N], BF16, name="kT")
            v_sb = persist.tile([P, NMB * (HD + 1)], BF16, name="v_sb")
            nc.gpsimd.memset(v_sb[:], 1.0)  # ones column default
            with tc.tile_pool(name="mm_psum", bufs=3, space="PSUM") as mm_psum:
                # per n-block (= per source rank) so attention inputs become
                # ready progressively as each rank's y slab lands
                for nb in range(NNB):
                    for mb in range(4 * nb, 4 * nb + 4):
                        psv = mm_psum.tile([P, 512], F32, name="ps_mm")
                        for cc in range(CCH):
                            nc.tensor.matmul(
                                psv[:, 0:HD],
                                lhsT=y_full[:, cc, mb * P : (mb + 1) * P],
                                rhs=wv_s[:, cc],
                                start=(cc == 0), stop=(cc == CCH - 1),
                            )
                        nc.vector.tensor_copy(
                            v_sb[:, mb * (HD + 1) : mb * (HD + 1) + HD],
                            psv[:, 0:HD],
                        )
                    psq = mm_psum.tile([P, 512], F32, name="ps_mm")
                    for cc in range(CCH):
                        nc.tensor.matmul(
                            psq, lhsT=wq_s[:, cc],
                            rhs=y_full[:, cc, nb * 512 : (nb + 1) * 512],
                            start=(cc == 0), stop=(cc == CCH - 1),
                        )
                    nc.scalar.activation(
                        qT[:, nb * 512 : (nb + 1) * 512], psq, AF.Identity,
                        bias=bq_eff, scale=1.0,
                    )
                    psk = mm_psum.tile([P, 512], F32, name="ps_mm")
                    for cc in range(CCH):
                        nc.tensor.matmul(
                            psk, lhsT=wk_s[:, cc],
                            rhs=y_full[:, cc, nb * 512 : (nb + 1) * 512],
                            start=(cc == 0), stop=(cc == CCH - 1),
                        )
                    nc.vector.tensor_copy(kT[:, nb * 512 : (nb + 1) * 512], psk)

            # ---- phase 4: attention ----
            # Per n-block b: scoresT[m, n] in 3-chunk packs ([128, 1536] psum,
            # 3 row-tiled K=32 matmuls on rotating 32-row bands) -> exp split
            # ACT (exact) / DVE (Schraudolph int16 bitcast) -> AV accumulated
            # in psum with m-chunk parity col-packed to partitions 0-32/64-96
            # (row 32 = sum of exp = softmax denominator via the ones column).
            numer4 = persist.tile([P, 2, 512], F32, name="numer4")
            den4 = persist.tile([P, 2, 512], F32, name="den4")
            recip4 = persist.tile([P, 2, 512], F32, name="recip4")
            nc.gpsimd.memset(den4[:], 1.0)
            out4 = persist.tile([P, 2, 512], BF16, name="out4")
            bcast_sb = persist.tile([P, 512], F32, name="bcast_sb")
            a2a_in = dram.tile([NCORES, HD, NS], BF16, name="a2a_in")
            a2a_out = dram.tile([NCORES, HD, NS], BF16, name="a2a_out")
            bv4 = persist.tile([P, 1], F32, name="bv4")
            for i in range(4):
                nc.vector.tensor_copy(bv4[32 * i : 32 * i + 32], bv_eff)

            # pack layout: chunks [3p, 3p+1, 3p+2] (last pack has 2);
            # greedy ACT/DVE split at 512-col granularity per pack
            packs = [list(range(i, min(i + 3, NMB))) for i in range(0, NMB, 3)]
            # per-op costs fit from hardware traces (dur = NS0 + NSC*cols)
            ACT_NS0, ACT_NSC = 312.0, 1.0
            DVE_NS0, DVE_NSC = 191.0, 1.246
            DVE_BLK_FIXED = 2900.0  # per-block evac/combine/norm on DVE

            with tc.tile_pool(name="sc_psum", bufs=2, space="PSUM") as sc_psum, \
                 tc.tile_pool(name="av_psum", bufs=2, space="PSUM") as av_psum, \
                 tc.tile_pool(name="exp_pool", bufs=6) as exp_pool:
                band = 0
                act_ns, dve_ns = 0.0, 0.0
                av_t = {}        # block -> av psum tile
                exd = {}         # block -> {pack_idx: ex tile}
                next_pair = {}   # block -> next uncovered m-chunk pair

                def finish_block(b):
                    # evacuate + combine halves (frees the av bank quickly);
                    # a DVE op may read at most ONE psum operand -> stage the
                    # second column band through SBUF first
                    av = av_t.pop(b)
                    exd.pop(b)
                    bh = 32 * (b % 4)
                    g = b // 4
                    tmp33 = work.tile([HD + 1, 512], F32, name="tmp33")
                    nc.vector.tensor_copy(tmp33, av[64 : 64 + HD + 1, :])
                    nc.vector.tensor_tensor(
                        numer4[bh : bh + HD, g, :],
                        av[0:HD, :], tmp33[0:HD, :], ALU.add,
                    )
                    nc.vector.tensor_tensor(
                        den4[bh : bh + 1, g, :],
                        av[HD : HD + 1, :], tmp33[HD : HD + 1, :], ALU.add,
                    )
                    if b % 4 == 3:
                        # normalize this group of 4 blocks (group 0 overlaps
                        # with blocks 4-7 compute); denominator rows sit at
                        # quadrant leaders {0,32,64,96}: fast-approx
                        # reciprocal (1 DVE op, ~18 bits — plenty here),
                        # broadcast + multiply + bias on DVE.
                        g_ = b // 4
                        nc.vector.reciprocal_approx_fast(
                            recip4[:, g_, :], den4[:, g_, :]
                        )
                        nc.vector.stream_shuffle(
                            bcast_sb[:], recip4[:, g_, :], mask=[0] * 32
                        )
                        nc.vector.tensor_tensor(
                            out4[:, g_, :], numer4[:, g_, :], bcast_sb, ALU.mult
                        )
                        # group 0's bias add can go to the idle GPSIMD (pure
                        # SBUF op, consumed ~40us later); group 1 feeds the
                        # AllToAll at the tail where the faster DVE wins
                        (nc.gpsimd if g_ == 0 else nc.vector).tensor_scalar_add(
                            out4[:, g_, :], out4[:, g_, :], bv4
                        )
                        for j in range(4 * g_, 4 * g_ + 4):
                            nc.sync.dma_start(
                                a2a_in[j],
                                out4[32 * (j % 4) : 32 * (j % 4) + HD, j // 4, :],
                            )

                def emit_tail(b, pi, pk, sc):
                    # exp + AV for a pack whose scores were emitted one
                    # iteration earlier (software pipelining: the next pack's
                    # score matmuls sit AHEAD of these AV matmuls in PE
                    # program order, so the PE streams scores while ACT/DVE
                    # do this pack's exp)
                    nonlocal act_ns, dve_ns
                    if pi == 0:
                        av_t[b] = av_psum.tile([97, 512], F32, name="av")
                        exd[b] = {}
                        next_pair[b] = 0
                        dve_ns += DVE_BLK_FIXED
                    ncols = 512 * len(pk)
                    # choose ACT's share (bank-aligned) to balance engines
                    best, best_a = None, ncols
                    for a in range(0, ncols + 1, 512):
                        fin = max(
                            act_ns + (ACT_NS0 + a * ACT_NSC if a else 0.0),
                            dve_ns + (DVE_NS0 + (ncols - a) * DVE_NSC
                                      if a < ncols else 0.0),
                        )
                        if best is None or fin < best:
                            best, best_a = fin, a
                    a = best_a
                    # SEPARATE output tiles per engine: a shared tile would
                    # WAW-serialize the DVE piece behind the ACT piece, which
                    # delays freeing the score psum bank and bubbles the pipe
                    ex_a = ex_d = None
                    if a > 0:
                        ex_a = exp_pool.tile([P, 1536], BF16, name="ex_a")
                        nc.scalar.activation(ex_a[:, 0:a], sc[:, 0:a], AF.Exp)
                        act_ns += ACT_NS0 + a * ACT_NSC
                    if a < ncols:
                        ex_d = exp_pool.tile([P, 1536], BF16, name="ex_d")
                        exi = ex_d[:, 0 : ncols - a].bitcast(I16)
                        nc.vector.tensor_scalar(
                            exi, sc[:, a:ncols], SCH_A, SCH_B,
                            op0=ALU.mult, op1=ALU.add,
                        )
                        dve_ns += DVE_NS0 + (ncols - a) * DVE_NSC
                    exd[b][pi] = (ex_a, ex_d, a)
                    # AV for every m-chunk pair fully covered by now
                    av = av_t[b]
                    done = pk[-1]
                    while next_pair[b] * 2 + 1 <= done:
                        for cm in (2 * next_pair[b], 2 * next_pair[b] + 1):
                            rr = cm % 2
                            pea, ped, pa = exd[b][cm // 3]
                            off = (cm % 3) * 512
                            if off < pa:
                                src = pea[:, off : off + 512]
                            else:
                                src = ped[:, off - pa : off - pa + 512]
                            nc.tensor.matmul(
                                av[64 * rr : 64 * rr + HD + 1, :],
                                lhsT=v_sb[:, cm * (HD + 1) : (cm + 1) * (HD + 1)],
                                rhs=src,
                                start=(cm < 2),
                                stop=(cm >= NMB - 2),
                                tile_position=(0, 64 * rr),
                                skip_group_check=True,
                            )
                        next_pair[b] += 1
                    if pi == len(packs) - 1:
                        finish_block(b)

                pend = None
                for b in range(NNB):
                    for pi, pk in enumerate(packs):
                        sc = sc_psum.tile([P, 1536], F32, name="sc")
                        for j, cm in enumerate(pk):
                            nc.tensor.matmul(
                                sc[:, j * 512 : (j + 1) * 512],
                                lhsT=kT[32 * band : 32 * band + 32,
                                        cm * P : (cm + 1) * P],
                                rhs=qT[32 * band : 32 * band + 32,
                                       b * 512 : (b + 1) * 512],
                                start=True, stop=True,
                                tile_position=(32 * band, 0),
                            )
                            band = (band + 1) % 4
                        if pend is not None:
                            emit_tail(*pend)
                        pend = (b, pi, pk, sc)
                emit_tail(*pend)

            # ---- phase 5: all-to-all + projection ----
            nc.gpsimd.collective_compute(
                "AllToAll",
                ALU.bypass,
                replica_groups=[list(range(NCORES))],
                ins=[a2a_in[:].opt()],
                outs=[a2a_out[:].opt()],
            )
            # keep the PE busy through the all-to-all so HAM doesn't
            # re-throttle the clock before the projection matmuls
            with tc.tile_pool(name="warm2_psum", bufs=1, space="PSUM") as warm2:
                wps2 = warm2.tile([P, 256], F32, name="wps2")
                for _ in range(40):
                    nc.tensor.matmul(
                        wps2, lhsT=warm_in[:, 0:P], rhs=warm_in[:],
                        start=True, stop=True,
                    )
            cat = a2a_out[:].rearrange("h d f -> (h d) f")  # [256, 512]
            c_sb = persist.tile([P, CCH, NS], BF16, name="c_sb")
            out_sb = persist.tile([P, CCH, NS], F32, name="out_sb")
            with tc.tile_pool(name="proj_psum", bufs=2, space="PSUM") as proj_psum:
                # split spatial in half so the second half's DMA overlaps the
                # first half's matmuls (the PE is cold here; keep it short)
                for h2 in range(2):
                    s0, s1 = h2 * 256, (h2 + 1) * 256
                    for cc in range(CCH):
                        nc.sync.dma_start(
                            c_sb[:, cc, s0:s1], cat[cc * P : (cc + 1) * P, s0:s1]
                        )
                    for ob in range(CCH):
                        psp = proj_psum.tile([P, 256], F32, name="ps_proj")
                        for cc in range(CCH):
                            nc.tensor.matmul(
                                psp,
                                lhsT=wproj_sb[:, cc, ob * P : (ob + 1) * P],
                                rhs=c_sb[:, cc, s0:s1],
                                start=(cc == 0), stop=(cc == CCH - 1),
                            )
                        nc.scalar.activation(
                            out_sb[:, ob, s0:s1], psp, AF.Identity,
                            bias=bproj_sb[:, ob : ob + 1], scale=1.0,
                        )
                        nc.sync.dma_start(out_d[ob, :, s0:s1], out_sb[:, ob, s0:s1])

    nc.compile()
    return nc


def _host_prep(x, w_dw, b_dw, w_qkv, b_qkv, w_proj, b_proj):
    """Build per-core in_maps from full inputs."""
    x = np.asarray(x, np.float32)
    w_dw = np.asarray(w_dw, np.float32)
    b_dw = np.asarray(b_dw, np.float32)
    w_qkv = np.asarray(w_qkv, np.float32)
    b_qkv = np.asarray(b_qkv, np.float32)
    w_proj = np.asarray(w_proj, np.float32)
    b_proj = np.asarray(b_proj, np.float32)

    xs = x[0]  # [C, 16, 16, 16]
    # diag conv weights: [CCH, 27, P, P]
    dw_diag = np.zeros((CCH, 27, P, P), np.float32)
    for cc in range(CCH):
        for t, (dh, dw_, dd) in enumerate(TAPS):
            np.fill_diagonal(dw_diag[cc, t], w_dw[cc * P : (cc + 1) * P, 0, dh, dw_, dd])
    dw_diag = dw_diag.astype(bf16)
    b_dw_s = b_dw.reshape(CCH, P, 1)

    wproj_t = np.ascontiguousarray(w_proj.T).reshape(CCH, P, C).astype(bf16)
    bproj_s = b_proj.reshape(CCH, P, 1)

    in_maps = []
    for h in range(NCORES):
        # padded x slab: global h rows 2h-1 .. 2h+2, padded w/d
        xp = np.zeros((C, 4, 18, 18), np.float32)
        for hl in range(4):
            hg = 2 * h - 1 + hl
            if 0 <= hg < HWD:
                xp[:, hl, 1:17, 1:17] = xs[:, hg]
        xp = xp.reshape(CCH, P, 4 * 18 * 18).astype(bf16)

        # q weights/bias carry the 1/sqrt(hd) scale from the host
        wq_h = w_qkv[h * HD : (h + 1) * HD] * SCALE  # [32, 256]
        wk_h = w_qkv[C + h * HD : C + (h + 1) * HD]
        wv_h = w_qkv[2 * C + h * HD : 2 * C + (h + 1) * HD]
        wq_rep = np.tile(wq_h.T, (1, 4)).reshape(C, P)  # [256, 128]
        wk_rep = np.tile(wk_h.T, (1, 4)).reshape(C, P)
        in_maps.append({
            "x_pad": xp,
            "dw_diag": dw_diag,
            "b_dw": b_dw_s,
            "wq": wq_rep.reshape(CCH, P, P).astype(bf16),
            "wk": wk_rep.reshape(CCH, P, P).astype(bf16),
            "wv": np.ascontiguousarray(wv_h.T).reshape(CCH, P, HD).astype(bf16),
            "bq": (np.tile(b_qkv[h * HD : (h + 1) * HD], 4) * SCALE)
                  .reshape(P, 1).astype(np.float32),
            "bv": b_qkv[2 * C + h * HD : 2 * C + (h + 1) * HD].reshape(HD, 1).astype(np.float32),
            "wproj": wproj_t,
            "bproj": bproj_s,
        })
    return in_maps


def kernel(**inputs):
    if "nc" not in _cache:
        _cache["nc"] = _build_graph()
    nc = _cache["nc"]
    in_maps = _host_prep(**inputs)
    res = bass_utils.run_bass_kernel_spmd(nc, in_maps, core_ids=list(range(NCORES)))
    slices = [res.results[j]["out"].reshape(C, NS) for j in range(NCORES)]
    full = np.concatenate(slices, axis=1)  # [256, 4096]
    return full.reshape(1, C, HWD, HWD, HWD).astype(np.float32)


if __name__ == "__main__":
    nc = _build_graph()
    print("graph built + compiled OK")
